# revision 30
# baseline (speedup 1.0000x reference)
"""BatchCenterLoss Trainium2 kernel (8 NeuronCores, SPMD via bass_utils).

Loss = sum over same-class pairs (i != j) of ||x_i - x_j|| / 2 / B.

Strategy -- class-sharded data-parallel with HOST-side preprocessing:
the host argsorts rows by class, assigns classes to cores (balancing
per-core slot widths so every core runs the same SPMD slot pattern),
gathers + transposes each core's rows into xgT [D=128, W] (bf16), and
precomputes row norms + pad penalties. The device then only does the
O(B^2/classes) part per class slot b (padded width C_b, h0 = first 128
rows, h1 = remaining hw = C_b-128 rows, slot cols stored [h1|h0]):

  - PSUM [128, S=C_b+hw] accumulates, via pairs of bf16 matmuls
    (1 cyc/row on PE), p = g - 0.5*(n_i+pen_i) - 0.5*(n_j+pen_j):
      [T01|T00] = rows h0 x cols [h1|h0]   (one K=128 matmul)
      [T11ext]  = rows [h1|h0-prefix] x cols h1  (pen'd extension rows)
    plus one K=2 rank-2 matmul each for the norm/penalty terms
    (lhsT rows = [u_i, 1], rhs rows = [1, v_j], u=v=-0.5(n+pen)).
  - two strided DVE tensor_scalars per multi-slot relu group:
    t1 = min(p,0)*-2 = relu(dist^2+pens) on the diag cols (T00|T11) and
    min(p,0)*-8 on the off-diag T01 cols -- the *4 under the sqrt folds
    the x2 cross-pair weight in (sqrt(4d) = 2*dist), so every sqrt
    chunk is uniform. Pads/class-mismatch rows see -1e9 penalties and
    die in the relu; the i==i diagonal is left in (relu'd fp roundoff
    contributes ~1e-5 relative after sqrt, far below tolerance).
  - one flat Sqrt+accum_out instruction on ACT per group (amortizes the
    ~370ns per-instruction ACT overhead); accum_out -> rs column.
rs is DMA'd out; the host sums in float64 and scales by 1/(2B).

Cost-model-informed choices (TimelineSim is the timing metric here):
  - bf16 matmuls run 1 cycle/row vs fp32's 4 (PSUM accum stays fp32;
    norms come from bf16-rounded x on host so the diagonal cancels).
  - a t=0 pixel matmul on a memset tile starts the PE p-state ramp so
    real matmuls hit 2.4 GHz; a t=0 dummy Sqrt preloads the ACT table
    during the DMA head.
  - indirect DMA (SWDGE descriptor gen ~1us/tile on Pool) is avoided
    entirely by the host-side gather; inputs arrive as 3 wide HWDGE
    DMAs + 1 aux DMA.
"""

from contextlib import ExitStack

import numpy as np
import ml_dtypes

import concourse.bass as bass
import concourse.tile as tile
from concourse import bacc, mybir
from concourse.bass_utils import run_bass_kernel_spmd

B = 16384
D = 128
NCLS = 100
NCORES = 8
NSLOT = (NCLS + NCORES - 1) // NCORES  # 13

F32 = mybir.dt.float32
BF16 = mybir.dt.bfloat16
BF16_NP = ml_dtypes.bfloat16

PEN = -1.0e9

_prog_cache = {}
TRACE = False
LAST_RESULTS = None
LAST_NC = None

# schedule tunables (validated via TimelineSim sweeps)
GROUP_SIZES = [1, 2, 2, 4, 4]  # relu/sqrt group plan (slots per group)
OFF_ON_ACT = 0  # first N groups' off-diag relu runs on ACT (fills its idle head)
PSTP_BUFS = 2
PST1_BUFS = 2
SP_SPLITS = [(0, 1), None, (1, 3), (5, 7), (9, 11)]  # None = aux
POOL_SPLITS = [(11, 13), (3, 5), (7, 9)]


def _width_of(cnt):
    if cnt <= 128:
        return 128
    if cnt <= 192:
        return 192
    assert cnt <= 256, f"class too large: {cnt}"
    return 256


def _plan(counts):
    """Assign classes to cores; return (per-core class lists, slot width
    pattern). All cores share the same sorted-desc width pattern (SPMD)."""
    w = np.array([_width_of(int(c)) for c in counts])
    order_cls = sorted(range(NCLS), key=lambda c: (-w[c], -counts[c]))
    cores = [[] for _ in range(NCORES)]
    loads = np.zeros(NCORES)
    for c in order_cls:
        k = min(
            (kk for kk in range(NCORES) if len(cores[kk]) < NSLOT),
            key=lambda kk: loads[kk],
        )
        cores[k].append(c)
        loads[k] += w[c]
    for k in range(NCORES):
        while len(cores[k]) < NSLOT:
            cores[k].append(-1)
        cores[k].sort(key=lambda c: -(w[c] if c >= 0 else 128))
    pattern = tuple(
        max((w[cores[k][i]] if cores[k][i] >= 0 else 128) for k in range(NCORES))
        for i in range(NSLOT)
    )
    return cores, pattern


def _relu_groups(pattern):
    """Group consecutive same-width slots into relu groups whose PSUM
    tile spans <= 1024 fp32 cols (2 banks); matmul outputs never cross
    a 2KB bank boundary within these layouts. GROUP_SIZES is the target
    plan; groups always break at width changes and the 1024 cap."""
    sizes = list(GROUP_SIZES)
    groups = []
    cur, cols = [], 0
    si = 0

    def tgt():
        return sizes[si] if si < len(sizes) else sizes[-1]

    for b in range(NSLOT):
        S = 2 * pattern[b] - 128
        if cur and (
            cols + S > 1024
            or pattern[b] != pattern[cur[0]]
            or len(cur) >= tgt()
        ):
            groups.append(tuple(cur))
            si += 1
            cur, cols = [], 0
        cur.append(b)
        cols += S
    if cur:
        groups.append(tuple(cur))
    return groups


def _chunks_for_pattern(pattern, groups):
    """Sqrt chunks (uniform scale=1 now): one chunk per relu group --
    each issues as soon as its group's relu lands."""
    return [(g[0], g[-1] + 1) for g in groups]


def _layout(pattern):
    """Per-slot x-column and t1-column offsets."""
    xoff, toff = [], []
    xs = ts = 0
    for wdt in pattern:
        hw = wdt - 128
        xoff.append(xs)
        toff.append(ts)
        xs += wdt
        ts += wdt + hw
    return xoff, toff, xs, ts  # W (x cols), TS (t1 cols)


def _build(pattern):
    groups = _relu_groups(pattern)
    chunks = _chunks_for_pattern(pattern, groups)
    xoff, toff, W, TS = _layout(pattern)
    NCH = len(chunks)
    AW = 2 * W + 128 * NSLOT  # aux: [u|1] cols, [1|v] cols, T11ext lhsT cols

    nc = bacc.Bacc("TRN2", target_bir_lowering=False, debug=False)
    xgt = nc.dram_tensor("xgt", [128, W], BF16, kind="ExternalInput").ap()
    aux = nc.dram_tensor("aux", [2, AW], BF16, kind="ExternalInput").ap()
    outp = nc.dram_tensor("out", [128, NCH], F32, kind="ExternalOutput").ap()

    with ExitStack() as ctx:
        tc = ctx.enter_context(tile.TileContext(nc))
        const = ctx.enter_context(tc.tile_pool(name="const", bufs=1))
        pstp = ctx.enter_context(tc.tile_pool(name="pst", bufs=PSTP_BUFS, space="PSUM"))
        pst1 = ctx.enter_context(tc.tile_pool(name="pst1", bufs=PST1_BUFS, space="PSUM"))
        pswp = ctx.enter_context(tc.tile_pool(name="psw", bufs=1, space="PSUM"))

        xgt_sb = const.tile([128, W], BF16)
        aux_sb = const.tile([2, AW], BF16)
        t1 = const.tile([128, TS], F32)
        scratch = const.tile([128, 6 * 384], F32)
        rs = const.tile([128, NCH], F32)

        # t=0: start the PE p-state ramp + preload the Sqrt ACT table while
        # the first input DMAs are in flight.
        wz = const.tile([1, 16], BF16)
        nc.vector.memset(wz[:], 0.0)
        psw = pswp.tile([1, 16], F32)
        nc.tensor.matmul(out=psw[:], lhsT=wz[0:1, 0:1], rhs=wz[0:1, 0:16], start=True, stop=True)
        ds = const.tile([1, 8], F32)
        nc.vector.memset(ds[:], 1.0)
        dscr = const.tile([1, 8], F32)
        nc.scalar.activation(out=dscr[:], in_=ds[:], func=mybir.ActivationFunctionType.Sqrt)

        # input DMAs: x chunk0/chunk1 on SP (HWDGE); aux + later x chunks on
        # gpsimd (SWDGE) so the HWDGE holds don't serialize the head.
        def xcols(s0, s1):
            return xoff[s0], xoff[s1 - 1] + pattern[s1 - 1]

        # latest-needed chunk first on Pool: its first transfer reaches the
        # DMA engines before SP's queued aux does; don't put early slots there
        sp_splits = SP_SPLITS
        pool_splits = POOL_SPLITS
        for sp in sp_splits:
            if sp is None:
                nc.sync.dma_start(out=aux_sb[:], in_=aux)
                continue
            a, b = xcols(*sp)
            nc.sync.dma_start(out=xgt_sb[:, a:b], in_=xgt[:, a:b])
        for s0, s1 in pool_splits:
            a, b = xcols(s0, s1)
            nc.gpsimd.dma_start(out=xgt_sb[:, a:b], in_=xgt[:, a:b])

        rs_col = 0

        for gi, grp in enumerate(groups):
            k = len(grp)
            wdt = pattern[grp[0]]
            hw = wdt - 128
            S = wdt + hw
            gS = k * S
            ps = (pst1 if gS <= 512 else pstp).tile([128, 512 if gS <= 512 else 1024], F32)
            po = 0
            for b in grp:
                xo = xoff[b]
                h0 = xgt_sb[:, xo + hw : xo + wdt]
                # slot psum layout [T01 | T00 | T11]:
                # [T01|T00]: rows h0 x cols [h1|h0]
                nc.tensor.matmul(
                    out=ps[:, po : po + wdt], lhsT=h0,
                    rhs=xgt_sb[:, xo : xo + wdt],
                    start=True, stop=False,
                )
                nc.tensor.matmul(
                    out=ps[:, po : po + wdt],
                    lhsT=aux_sb[0:2, xo + hw : xo + wdt],
                    rhs=aux_sb[0:2, W + xo : W + xo + wdt],
                    start=False, stop=True,
                )
                if hw > 0:
                    # T11ext: rows [h1 | h0-prefix] x cols h1; extension rows
                    # are killed by the -0.5*PEN u values in the aux3 region.
                    nc.tensor.matmul(
                        out=ps[:, po + wdt : po + S],
                        lhsT=xgt_sb[:, xo : xo + 128],
                        rhs=xgt_sb[:, xo : xo + hw],
                        start=True, stop=False,
                    )
                    nc.tensor.matmul(
                        out=ps[:, po + wdt : po + S],
                        lhsT=aux_sb[0:2, 2 * W + b * 128 : 2 * W + (b + 1) * 128],
                        rhs=aux_sb[0:2, W + xo : W + xo + hw],
                        start=False, stop=True,
                    )
                po += S
            # relu: t1 = max(-2*p, 0) = relu(d^2 + pens); the off-diag T01
            # cols get -8 (=4x under the sqrt -> 2*dist pair weight), so the
            # later sqrt chunks are uniform-scale single instructions.
            t1g = t1[:, toff[grp[0]] : toff[grp[0]] + gS]
            if hw > 0:
                psr = ps[:, 0:gS].rearrange("p (b s) -> p b s", b=k, s=S)
                t1r = t1g.rearrange("p (b s) -> p b s", b=k, s=S)
                if gi < OFF_ON_ACT:
                    # off-diag relu on ACT: relu(-8*p) = 4*relu(d^2+pens)
                    nc.scalar.activation(
                        out=t1r[:, :, 0:hw], in_=psr[:, :, 0:hw],
                        func=mybir.ActivationFunctionType.Relu, scale=-8.0,
                    )
                else:
                    nc.vector.tensor_scalar(
                        out=t1r[:, :, 0:hw], in0=psr[:, :, 0:hw],
                        scalar1=0.0, scalar2=-8.0,
                        op0=mybir.AluOpType.min, op1=mybir.AluOpType.mult,
                    )
                nc.vector.tensor_scalar(
                    out=t1r[:, :, hw:S], in0=psr[:, :, hw:S],
                    scalar1=0.0, scalar2=-2.0,
                    op0=mybir.AluOpType.min, op1=mybir.AluOpType.mult,
                )
            else:
                nc.vector.tensor_scalar(
                    out=t1g, in0=ps[:, 0:gS],
                    scalar1=0.0, scalar2=-2.0,
                    op0=mybir.AluOpType.min, op1=mybir.AluOpType.mult,
                )
            # sqrt chunk for this group (uniform scale; accum -> one rs col)
            nc.scalar.activation(
                out=scratch[:, 0:gS], in_=t1g,
                func=mybir.ActivationFunctionType.Sqrt,
                accum_out=rs[:, rs_col : rs_col + 1],
            )
            rs_col += 1

        assert rs_col == NCH
        nc.sync.dma_start(out=outp[:, :], in_=rs[:])

    nc.compile()
    return nc


def _prep_inputs(x, target, cores, pattern):
    xoff, toff, W, TS = _layout(pattern)
    AW = 2 * W + 128 * NSLOT

    t = np.asarray(target).astype(np.int64).ravel()
    order = np.argsort(t, kind="stable").astype(np.int64)
    counts = np.bincount(t, minlength=NCLS)
    starts = np.concatenate([[0], np.cumsum(counts)])

    xb = np.asarray(x, dtype=np.float32).astype(BF16_NP)
    xd = xb.astype(np.float64)
    n = (xd * xd).sum(1)  # norms of the bf16-rounded rows (matches device g)

    in_maps = []
    for core in range(NCORES):
        gidx = np.zeros(W, dtype=np.int64)
        pen = np.full(W, PEN, dtype=np.float64)
        aux3_u = np.full(128 * NSLOT, -0.5 * PEN, dtype=np.float64)
        for b, cls in enumerate(cores[core]):
            wdt = pattern[b]
            hw = wdt - 128
            xo = xoff[b]
            if cls < 0:
                continue
            cnt = int(counts[cls])
            rows = order[starts[cls] : starts[cls] + cnt]
            # slot layout [h1|h0]: first hw cols = class rows 128..cnt,
            # next 128 cols = class rows 0..128
            n1 = max(0, cnt - 128)
            gidx[xo : xo + n1] = rows[128 : 128 + n1]
            pen[xo : xo + n1] = 0.0
            n0 = min(cnt, 128)
            gidx[xo + hw : xo + hw + n0] = rows[:n0]
            pen[xo + hw : xo + hw + n0] = 0.0
            if hw > 0:
                # T11ext lhsT: u for h1 rows, PEN for the extension rows
                u3 = np.full(128, -0.5 * PEN, dtype=np.float64)
                u3[:n1] = -0.5 * (n[rows[128 : 128 + n1]])
                aux3_u[b * 128 : (b + 1) * 128] = u3

        u = -0.5 * (n[gidx] + pen)
        auxh = np.zeros((2, AW), dtype=np.float64)
        auxh[0, 0:W] = u
        auxh[1, 0:W] = 1.0
        auxh[0, W : 2 * W] = 1.0
        auxh[1, W : 2 * W] = u
        auxh[0, 2 * W :] = aux3_u
        auxh[1, 2 * W :] = 1.0

        in_maps.append(
            {
                "xgt": np.ascontiguousarray(xb[gidx].T),
                "aux": auxh.astype(BF16_NP),
            }
        )
    return in_maps


def kernel(x, target):
    t = np.asarray(target).astype(np.int64).ravel()
    counts = np.bincount(t, minlength=NCLS)
    cores, pattern = _plan(counts)
    if pattern not in _prog_cache:
        _prog_cache[pattern] = _build(pattern)
    nc = _prog_cache[pattern]
    global LAST_RESULTS, LAST_NC
    LAST_NC = nc
    in_maps = _prep_inputs(x, target, cores, pattern)
    results = run_bass_kernel_spmd(nc, in_maps, list(range(NCORES)), trace=TRACE)
    LAST_RESULTS = results
    total = float(
        sum(np.asarray(r["out"], dtype=np.float64).sum() for r in results.results)
    )
    return np.float32(total / 2.0 / B)


# revision 35
# speedup vs baseline: 1.0280x; 1.0280x over previous
"""BatchCenterLoss Trainium2 kernel (8 NeuronCores, SPMD via bass_utils).

Loss = sum over same-class pairs (i != j) of ||x_i - x_j|| / 2 / B.

Strategy -- class-sharded data-parallel with HOST-side preprocessing:
the host argsorts rows by class, assigns classes to cores (balancing
per-core slot widths so every core runs the same SPMD slot pattern),
gathers + transposes each core's rows into xgT [D=128, W] (bf16), and
precomputes row norms + pad penalties. The device then only does the
O(B^2/classes) part per class slot b (padded width C_b, h0 = first 128
rows, h1 = remaining hw = C_b-128 rows, slot cols stored [h1|h0]):

  - PSUM [128, S=C_b+hw] accumulates, via pairs of bf16 matmuls
    (1 cyc/row on PE), p = g - 0.5*(n_i+pen_i) - 0.5*(n_j+pen_j):
      [T01|T00] = rows h0 x cols [h1|h0]   (one K=128 matmul)
      [T11ext]  = rows [h1|h0-prefix] x cols h1  (pen'd extension rows)
    plus one K=2 rank-2 matmul each for the norm/penalty terms
    (lhsT rows = [u_i, 1], rhs rows = [1, v_j], u=v=-0.5(n+pen)).
  - two strided DVE tensor_scalars per multi-slot relu group:
    t1 = min(p,0)*-2 = relu(dist^2+pens) on the diag cols (T00|T11) and
    min(p,0)*-8 on the off-diag T01 cols -- the *4 under the sqrt folds
    the x2 cross-pair weight in (sqrt(4d) = 2*dist), so every sqrt
    chunk is uniform. Pads/class-mismatch rows see -1e9 penalties and
    die in the relu; the i==i diagonal is left in (relu'd fp roundoff
    contributes ~1e-5 relative after sqrt, far below tolerance).
  - one flat Sqrt+accum_out instruction on ACT per group (amortizes the
    ~370ns per-instruction ACT overhead); accum_out -> rs column.
rs is DMA'd out; the host sums in float64 and scales by 1/(2B).

Cost-model-informed choices (TimelineSim is the timing metric here):
  - bf16 matmuls run 1 cycle/row vs fp32's 4 (PSUM accum stays fp32;
    norms come from bf16-rounded x on host so the diagonal cancels).
  - a t=0 pixel matmul on a memset tile starts the PE p-state ramp so
    real matmuls hit 2.4 GHz; a t=0 dummy Sqrt preloads the ACT table
    during the DMA head.
  - indirect DMA (SWDGE descriptor gen ~1us/tile on Pool) is avoided
    entirely by the host-side gather; inputs arrive as 3 wide HWDGE
    DMAs + 1 aux DMA.
"""

from contextlib import ExitStack

import numpy as np
import ml_dtypes

import concourse.bass as bass
import concourse.tile as tile
from concourse import bacc, mybir
from concourse.bass_utils import run_bass_kernel_spmd

B = 16384
D = 128
NCLS = 100
NCORES = 8
NSLOT = (NCLS + NCORES - 1) // NCORES  # 13

F32 = mybir.dt.float32
BF16 = mybir.dt.bfloat16
BF16_NP = ml_dtypes.bfloat16

PEN = -1.0e9

_prog_cache = {}
TRACE = False
LAST_RESULTS = None
LAST_NC = None

# schedule tunables (validated via TimelineSim sweeps)
GROUP_SIZES = [1, 2, 2, 4, 6]  # relu/sqrt group plan (slots per group)
OFF_ON_ACT = 0  # first N groups' off-diag relu runs on ACT (fills its idle head)
BIG_LAST = False  # sort slots ascending by width (big class last -> small tail)
PSTP_BUFS = 2
PST1_BUFS = 2
SP_SPLITS = [(0, 1), None, (1, 3), (5, 7), (9, 11)]  # None = aux
POOL_SPLITS = [(11, 13), (3, 5), (7, 9)]


def _width_of(cnt):
    assert cnt <= 256, f"class too large: {cnt}"
    return max(128, ((int(cnt) + 31) // 32) * 32)


def _plan(counts):
    """Assign classes to cores; return (per-core class lists, slot width
    pattern). All cores share the same sorted-desc width pattern (SPMD)."""
    w = np.array([_width_of(int(c)) for c in counts])
    order_cls = sorted(range(NCLS), key=lambda c: (-w[c], -counts[c]))
    cores = [[] for _ in range(NCORES)]
    loads = np.zeros(NCORES)
    for c in order_cls:
        k = min(
            (kk for kk in range(NCORES) if len(cores[kk]) < NSLOT),
            key=lambda kk: loads[kk],
        )
        cores[k].append(c)
        loads[k] += w[c]
    for k in range(NCORES):
        while len(cores[k]) < NSLOT:
            cores[k].append(-1)
        cores[k].sort(
            key=lambda c: (counts[c] if c >= 0 else 0) * (1 if BIG_LAST else -1)
        )
    # per-position width = max over cores; sorting by count aligns the
    # order statistics so same-position classes have similar sizes
    pattern = tuple(
        max((w[cores[k][i]] if cores[k][i] >= 0 else 128) for k in range(NCORES))
        for i in range(NSLOT)
    )
    return cores, pattern


def _relu_groups(pattern):
    """Group consecutive same-width slots into relu groups whose PSUM
    tile spans <= 1024 fp32 cols (2 banks); matmul outputs never cross
    a 2KB bank boundary within these layouts. GROUP_SIZES is the target
    plan; groups always break at width changes and the 1024 cap."""
    sizes = list(GROUP_SIZES)
    groups = []
    cur, cols = [], 0
    si = 0

    def tgt():
        return sizes[si] if si < len(sizes) else sizes[-1]

    for b in range(NSLOT):
        S = 2 * pattern[b] - 128
        if cur and (
            cols + S > 1024
            or pattern[b] != pattern[cur[0]]
            or len(cur) >= tgt()
        ):
            groups.append(tuple(cur))
            si += 1
            cur, cols = [], 0
        cur.append(b)
        cols += S
    if cur:
        groups.append(tuple(cur))
    return groups


def _chunks_for_pattern(pattern, groups):
    """Sqrt chunks (uniform scale=1 now): one chunk per relu group --
    each issues as soon as its group's relu lands."""
    return [(g[0], g[-1] + 1) for g in groups]


def _layout(pattern):
    """Per-slot x-column and t1-column offsets."""
    xoff, toff = [], []
    xs = ts = 0
    for wdt in pattern:
        hw = wdt - 128
        xoff.append(xs)
        toff.append(ts)
        xs += wdt
        ts += wdt + hw
    return xoff, toff, xs, ts  # W (x cols), TS (t1 cols)


def _build(pattern):
    groups = _relu_groups(pattern)
    chunks = _chunks_for_pattern(pattern, groups)
    xoff, toff, W, TS = _layout(pattern)
    NCH = len(chunks)
    AW = 2 * W + 128 * NSLOT  # aux: [u|1] cols, [1|v] cols, T11ext lhsT cols

    nc = bacc.Bacc("TRN2", target_bir_lowering=False, debug=False)
    xgt = nc.dram_tensor("xgt", [128, W], BF16, kind="ExternalInput").ap()
    aux = nc.dram_tensor("aux", [2, AW], BF16, kind="ExternalInput").ap()
    outp = nc.dram_tensor("out", [128, NCH], F32, kind="ExternalOutput").ap()

    with ExitStack() as ctx:
        tc = ctx.enter_context(tile.TileContext(nc))
        const = ctx.enter_context(tc.tile_pool(name="const", bufs=1))
        pstp = ctx.enter_context(tc.tile_pool(name="pst", bufs=PSTP_BUFS, space="PSUM"))
        pst1 = ctx.enter_context(tc.tile_pool(name="pst1", bufs=PST1_BUFS, space="PSUM"))
        pswp = ctx.enter_context(tc.tile_pool(name="psw", bufs=1, space="PSUM"))

        xgt_sb = const.tile([128, W], BF16)
        aux_sb = const.tile([2, AW], BF16)
        t1 = const.tile([128, TS], F32)
        scratch = const.tile([128, 6 * 384], F32)
        rs = const.tile([128, NCH], F32)

        # t=0: start the PE p-state ramp + preload the Sqrt ACT table while
        # the first input DMAs are in flight.
        wz = const.tile([1, 16], BF16)
        nc.vector.memset(wz[:], 0.0)
        psw = pswp.tile([1, 16], F32)
        nc.tensor.matmul(out=psw[:], lhsT=wz[0:1, 0:1], rhs=wz[0:1, 0:16], start=True, stop=True)
        ds = const.tile([1, 8], F32)
        nc.vector.memset(ds[:], 1.0)
        dscr = const.tile([1, 8], F32)
        nc.scalar.activation(out=dscr[:], in_=ds[:], func=mybir.ActivationFunctionType.Sqrt)

        # input DMAs: x chunk0/chunk1 on SP (HWDGE); aux + later x chunks on
        # gpsimd (SWDGE) so the HWDGE holds don't serialize the head.
        def xcols(s0, s1):
            return xoff[s0], xoff[s1 - 1] + pattern[s1 - 1]

        # latest-needed chunk first on Pool: its first transfer reaches the
        # DMA engines before SP's queued aux does; don't put early slots there
        sp_splits = SP_SPLITS
        pool_splits = POOL_SPLITS
        for sp in sp_splits:
            if sp is None:
                nc.sync.dma_start(out=aux_sb[:], in_=aux)
                continue
            a, b = xcols(*sp)
            nc.sync.dma_start(out=xgt_sb[:, a:b], in_=xgt[:, a:b])
        for s0, s1 in pool_splits:
            a, b = xcols(s0, s1)
            nc.gpsimd.dma_start(out=xgt_sb[:, a:b], in_=xgt[:, a:b])

        rs_col = 0

        for gi, grp in enumerate(groups):
            k = len(grp)
            wdt = pattern[grp[0]]
            hw = wdt - 128
            S = wdt + hw
            gS = k * S
            ps = (pst1 if gS <= 512 else pstp).tile([128, 512 if gS <= 512 else 1024], F32)
            po = 0
            for b in grp:
                xo = xoff[b]
                h0 = xgt_sb[:, xo + hw : xo + wdt]
                # slot psum layout [T01 | T00 | T11]:
                # [T01|T00]: rows h0 x cols [h1|h0]
                nc.tensor.matmul(
                    out=ps[:, po : po + wdt], lhsT=h0,
                    rhs=xgt_sb[:, xo : xo + wdt],
                    start=True, stop=False,
                )
                nc.tensor.matmul(
                    out=ps[:, po : po + wdt],
                    lhsT=aux_sb[0:2, xo + hw : xo + wdt],
                    rhs=aux_sb[0:2, W + xo : W + xo + wdt],
                    start=False, stop=True,
                )
                if hw > 0:
                    # T11ext: rows [h1 | h0-prefix] x cols h1; extension rows
                    # are killed by the -0.5*PEN u values in the aux3 region.
                    nc.tensor.matmul(
                        out=ps[:, po + wdt : po + S],
                        lhsT=xgt_sb[:, xo : xo + 128],
                        rhs=xgt_sb[:, xo : xo + hw],
                        start=True, stop=False,
                    )
                    nc.tensor.matmul(
                        out=ps[:, po + wdt : po + S],
                        lhsT=aux_sb[0:2, 2 * W + b * 128 : 2 * W + (b + 1) * 128],
                        rhs=aux_sb[0:2, W + xo : W + xo + hw],
                        start=False, stop=True,
                    )
                po += S
            # relu: t1 = max(-2*p, 0) = relu(d^2 + pens); the off-diag T01
            # cols get -8 (=4x under the sqrt -> 2*dist pair weight), so the
            # later sqrt chunks are uniform-scale single instructions.
            t1g = t1[:, toff[grp[0]] : toff[grp[0]] + gS]
            if hw > 0:
                psr = ps[:, 0:gS].rearrange("p (b s) -> p b s", b=k, s=S)
                t1r = t1g.rearrange("p (b s) -> p b s", b=k, s=S)
                if gi < OFF_ON_ACT:
                    # off-diag relu on ACT: relu(-8*p) = 4*relu(d^2+pens)
                    nc.scalar.activation(
                        out=t1r[:, :, 0:hw], in_=psr[:, :, 0:hw],
                        func=mybir.ActivationFunctionType.Relu, scale=-8.0,
                    )
                else:
                    nc.vector.tensor_scalar(
                        out=t1r[:, :, 0:hw], in0=psr[:, :, 0:hw],
                        scalar1=0.0, scalar2=-8.0,
                        op0=mybir.AluOpType.min, op1=mybir.AluOpType.mult,
                    )
                nc.vector.tensor_scalar(
                    out=t1r[:, :, hw:S], in0=psr[:, :, hw:S],
                    scalar1=0.0, scalar2=-2.0,
                    op0=mybir.AluOpType.min, op1=mybir.AluOpType.mult,
                )
            else:
                nc.vector.tensor_scalar(
                    out=t1g, in0=ps[:, 0:gS],
                    scalar1=0.0, scalar2=-2.0,
                    op0=mybir.AluOpType.min, op1=mybir.AluOpType.mult,
                )
            # sqrt chunk for this group (uniform scale; accum -> one rs col)
            nc.scalar.activation(
                out=scratch[:, 0:gS], in_=t1g,
                func=mybir.ActivationFunctionType.Sqrt,
                accum_out=rs[:, rs_col : rs_col + 1],
            )
            rs_col += 1

        assert rs_col == NCH
        nc.sync.dma_start(out=outp[:, :], in_=rs[:])

    nc.compile()
    return nc


def _prep_inputs(x, target, cores, pattern):
    xoff, toff, W, TS = _layout(pattern)
    AW = 2 * W + 128 * NSLOT

    t = np.asarray(target).astype(np.int64).ravel()
    order = np.argsort(t, kind="stable").astype(np.int64)
    counts = np.bincount(t, minlength=NCLS)
    starts = np.concatenate([[0], np.cumsum(counts)])

    xb = np.asarray(x, dtype=np.float32).astype(BF16_NP)
    xd = xb.astype(np.float64)
    n = (xd * xd).sum(1)  # norms of the bf16-rounded rows (matches device g)

    in_maps = []
    for core in range(NCORES):
        gidx = np.zeros(W, dtype=np.int64)
        pen = np.full(W, PEN, dtype=np.float64)
        aux3_u = np.full(128 * NSLOT, -0.5 * PEN, dtype=np.float64)
        for b, cls in enumerate(cores[core]):
            wdt = pattern[b]
            hw = wdt - 128
            xo = xoff[b]
            if cls < 0:
                continue
            cnt = int(counts[cls])
            rows = order[starts[cls] : starts[cls] + cnt]
            # slot layout [h1|h0]: first hw cols = class rows 128..cnt,
            # next 128 cols = class rows 0..128
            n1 = max(0, cnt - 128)
            gidx[xo : xo + n1] = rows[128 : 128 + n1]
            pen[xo : xo + n1] = 0.0
            n0 = min(cnt, 128)
            gidx[xo + hw : xo + hw + n0] = rows[:n0]
            pen[xo + hw : xo + hw + n0] = 0.0
            if hw > 0:
                # T11ext lhsT: u for h1 rows, PEN for the extension rows
                u3 = np.full(128, -0.5 * PEN, dtype=np.float64)
                u3[:n1] = -0.5 * (n[rows[128 : 128 + n1]])
                aux3_u[b * 128 : (b + 1) * 128] = u3

        u = -0.5 * (n[gidx] + pen)
        auxh = np.zeros((2, AW), dtype=np.float64)
        auxh[0, 0:W] = u
        auxh[1, 0:W] = 1.0
        auxh[0, W : 2 * W] = 1.0
        auxh[1, W : 2 * W] = u
        auxh[0, 2 * W :] = aux3_u
        auxh[1, 2 * W :] = 1.0

        in_maps.append(
            {
                "xgt": np.ascontiguousarray(xb[gidx].T),
                "aux": auxh.astype(BF16_NP),
            }
        )
    return in_maps


def kernel(x, target):
    t = np.asarray(target).astype(np.int64).ravel()
    counts = np.bincount(t, minlength=NCLS)
    cores, pattern = _plan(counts)
    if pattern not in _prog_cache:
        _prog_cache[pattern] = _build(pattern)
    nc = _prog_cache[pattern]
    global LAST_RESULTS, LAST_NC
    LAST_NC = nc
    in_maps = _prep_inputs(x, target, cores, pattern)
    results = run_bass_kernel_spmd(nc, in_maps, list(range(NCORES)), trace=TRACE)
    LAST_RESULTS = results
    total = float(
        sum(np.asarray(r["out"], dtype=np.float64).sum() for r in results.results)
    )
    return np.float32(total / 2.0 / B)
